# revision 2
# baseline (speedup 1.0000x reference)
"""Embedding-lookup v5: bf16 end-to-end on device, f32 upcast on host.

Same proven pipeline as the baseline (4-queue SWDGE non-transpose
gather HBM->SBUF, big contiguous SBUF->HBM writes), but the table and
the device output are bf16: 16.7 MB gather-read + 16.7 MB write per
core (vs 33.5 + 33.5).  W2 = W + b folded host-side then rounded once
to bf16 (rel err ~4e-3, tolerance 2e-2); kernel() upcasts the device
output to f32 on the host.
"""

from contextlib import ExitStack

import numpy as np

import concourse.mybir as mybir
from concourse import bacc, bass_utils, library_config
from concourse._compat import get_trn_type

B, T, D = 8192, 64, 128
NUM_DAYS = 365
N_CORES = 8
ROWS_PER_CORE = B // N_CORES            # 1024
N_IDX = ROWS_PER_CORE * T               # 65536 indices per core

G = 1024                                # indices per dma_gather call
NCALLS = N_IDX // G                     # 64
NBLK = G // 128                         # 8 rows per partition per call
G16 = G // 16
NBUF = 8                                # gather slot ring
NQUEUES = 4

_cache = {}


def _build_bass(reps=1):
    nc = bacc.Bacc(get_trn_type() or "TRN2", num_swdge_queues=NQUEUES)

    idx_l = nc.dram_tensor("idx_l", [NCALLS, 128, G16], mybir.dt.int16,
                           kind="ExternalInput")
    w = nc.dram_tensor("w", [NUM_DAYS, D], mybir.dt.bfloat16,
                       kind="ExternalInput")
    out = nc.dram_tensor("out", [N_IDX, D], mybir.dt.bfloat16,
                         kind="ExternalOutput")

    with ExitStack() as ctx:
        idx_sb = ctx.enter_context(
            nc.sbuf_tensor("idx_sb", [128, NCALLS, G16], mybir.dt.int16))
        g_sb = ctx.enter_context(
            nc.sbuf_tensor("g_sb", [128, NBUF, NBLK, D], mybir.dt.bfloat16))
        sem_idx = ctx.enter_context(nc.semaphore(name="sem_idx"))
        sem_g = [ctx.enter_context(nc.semaphore(name=f"sem_g{i}"))
                 for i in range(NBUF)]
        sem_out = [ctx.enter_context(nc.semaphore(name=f"sem_out{i}"))
                   for i in range(NBUF)]
        block = ctx.enter_context(nc.Block())

        total = reps * NCALLS

        @block.sync
        def _(sync):
            sync.dma_start(idx_sb[:],
                           idx_l[:].rearrange("t p g -> p t g")
                           ).then_inc(sem_idx, 16)
            for t in range(total):
                s, k = t % NBUF, t // NBUF
                sync.wait_ge(sem_g[s], 16 * (k + 1))
                tc = t % NCALLS
                out_ap = out[tc * G:(tc + 1) * G].rearrange(
                    "(p blk) d -> p blk d", p=128)
                sync.dma_start(out_ap, g_sb[:, s]).then_inc(sem_out[s], 16)
            for s in range(NBUF):
                n = total // NBUF + (total % NBUF > s)
                if n:
                    sync.wait_ge(sem_out[s], 16 * n)

        @block.gpsimd
        def _(gpsimd):
            gpsimd.load_library(library_config.mlp)
            gpsimd.wait_ge(sem_idx, 16)
            for t in range(total):
                s, k = t % NBUF, t // NBUF
                if t >= NBUF:
                    gpsimd.wait_ge(sem_out[s], 16 * k)
                gpsimd.dma_gather(
                    g_sb[:, s], w[:, :], idx_sb[:, t % NCALLS],
                    num_idxs=G, num_idxs_reg=G, elem_size=D,
                    queue_num=t % NQUEUES,
                    single_packet=False,
                ).then_inc(sem_g[s], 16)

    nc.compile()
    return nc


def _prep_idx(idx_core: np.ndarray) -> np.ndarray:
    """[N_IDX] int -> [NCALLS, 128, G16] int16 in dma_gather layout."""
    idx3 = idx_core.reshape(NCALLS, 128, NBLK).astype(np.int16)
    fed = idx3.transpose(0, 2, 1).reshape(NCALLS, G)
    wrap = fed.reshape(NCALLS, G16, 16).transpose(0, 2, 1)
    return np.ascontiguousarray(np.tile(wrap, (1, 8, 1)))


def _make_in_maps(batch_positions, W, b):
    import ml_dtypes
    w2 = (np.asarray(W, dtype=np.float32)
          + np.asarray(b, dtype=np.float32)[None, :])
    w2bf = np.ascontiguousarray(w2.astype(ml_dtypes.bfloat16))
    idx = np.asarray(batch_positions).reshape(B, T)
    in_maps = []
    for c in range(N_CORES):
        idx_core = idx[c * ROWS_PER_CORE:(c + 1) * ROWS_PER_CORE].reshape(-1)
        in_maps.append({"idx_l": _prep_idx(idx_core), "w": w2bf})
    return in_maps


def _run(batch_positions, W, b, trace=False):
    if "nc" not in _cache:
        _cache["nc"] = _build_bass()
    nc = _cache["nc"]
    in_maps = _make_in_maps(batch_positions, W, b)
    res = bass_utils.run_bass_kernel_spmd(
        nc, in_maps, core_ids=list(range(N_CORES)), trace=trace)
    out = np.empty((B, T, D), dtype=np.float32)
    for c in range(N_CORES):
        out[c * ROWS_PER_CORE:(c + 1) * ROWS_PER_CORE] = (
            np.asarray(res.results[c]["out"]).astype(np.float32)
            .reshape(ROWS_PER_CORE, T, D))
    return out, res


def kernel(**inputs) -> np.ndarray:
    out, _ = _run(inputs["batch_positions"], inputs["W"], inputs["b"])
    return out
